# revision 1
# baseline (speedup 1.0000x reference)
"""Trainium2 kernel for nn_KL_Loss (symmetric KL between triangular histograms
of Bayer phases).

Strategy
--------
Data-parallel over batch: core c handles batches {2c, 2c+1}. Each core gets 16
"slabs" of 512x512 fp32 pixels (2 local batches x 2 tensors (gt/out) x 4 Bayer
phases), sliced out host-side so all device DMA is contiguous.

Per slab the device computes a 1024-bin linear-interp histogram as two weighted
integer histograms via an outer-product decomposition on the TensorEngine:
bin = 32*hi + lo; for every group of 128 pixels (one SBUF column) build
U = onehot32(hi) (stationary) and V = [onehot32(lo) | onehot32(lo)*frac]
(moving), then accumulate PSUM[32, 64] += U^T @ V over all 2048 groups.
PSUM[:, :32] = per-bin counts C, PSUM[:, 32:] = per-bin frac sums S, and
hist_unnorm[t] = C[t] - S[t] + S[t-1].

The host gathers the 8x16 (C,S) pairs (small) and finishes the log/KL math in
float64 - the sharding hint's scalar all-reduce is this host-side gather.

floor() has no direct ALU op here; idx = round_to_nearest(x - 0.5) is used
instead. At exact-integer x either rounding direction yields the identical
histogram contribution ((1-frac)=0 case), and x=0 maps to idx=0/frac=0, so
this is exact, not approximate.
"""

import sys

sys.path.insert(0, "/opt/trn_rl_repo")

import numpy as np

import concourse.mybir as mybir
import concourse.tile as tile_mod
from concourse import bass
from concourse.bass_utils import run_bass_kernel_spmd
from concourse.vector_clock import ScopedClock

# ---------------------------------------------------------------------------
# Workaround for this walrus build: instructions carry at most one sync-wait
# command, but TileContext's exit path piles every outstanding proc-clock wait
# onto the single final Drain -> "Too many sync wait commands". Redistribute
# them one-per-nop before the drain.
# ---------------------------------------------------------------------------


def _split_drain_and_barrier(self, tick_clock, wait_clock):
    nc = self.nc
    collector = nc.sync.nop(nofuse=True, hint="drain_wait_split")
    wait_clock.add_sem_waits(
        collector.ins, ScopedClock({None: tick_clock.global_clock})
    )
    si = collector.ins.sync_info
    waits = list(si.on_wait) if si is not None else []
    if len(waits) > 1:
        collector.ins.sync_info = mybir.SyncInfo(
            on_wait=[waits[0]], on_update=list(si.on_update)
        )
        for w in waits[1:]:
            n = nc.sync.nop(nofuse=True, hint="drain_wait_split")
            n.ins.sync_info = mybir.SyncInfo(on_wait=[w], on_update=[])

    nc.sync.drain()
    nc.all_engine_barrier()
    assert self.sems is not None
    popped = nc._tile_sem_poison_stack.pop()
    assert popped is self._sem_poison
    nc.clear_and_free_semaphores(list(self.sems.allocated().values()))
    nc.all_engine_barrier()


tile_mod.TileContext._drain_and_barrier = _split_drain_and_barrier

# ---------------------------------------------------------------------------
# Problem constants (hardcoded per spec: inputs (16,1,1024,1024) fp32).
# ---------------------------------------------------------------------------
B = 16
H = W = 1024
NCORES = 8
B_PER_CORE = B // NCORES
NSLAB = B_PER_CORE * 2 * 4          # 16 slabs per core
PH, PW = H // 2, W // 2             # 512 x 512 per phase
NPIX = PH * PW                      # 262144 pixels per slab
P = 128
FREE = NPIX // P                    # 2048 pixel-groups (columns) per slab
G = 128                             # groups per one-hot build instruction
CHUNK = 1024                        # prep chunk along free axis
N_BINS = 1024

f32 = mybir.dt.float32
f16 = mybir.dt.float16
i32 = mybir.dt.int32

_program_cache = {}


ROT = 1  # PE column-quadrant rotation depth (overlaps LDWEIGHTS with MM)


def _build_program(rot=None):
    rot = ROT if rot is None else rot
    nc = bass.Bass()
    x_d = nc.declare_dram_parameter("x", [NSLAB, P, FREE], f32, isOutput=False)
    out_d = nc.declare_dram_parameter("out", [NSLAB, 32 * rot, 64], f32, isOutput=True)

    with tile_mod.TileContext(nc) as tc:
        with (
            tc.tile_pool(name="const", bufs=1) as cpool,
            tc.tile_pool(name="xin", bufs=2) as xpool,
            tc.tile_pool(name="work", bufs=2) as wpool,
            tc.tile_pool(name="build", bufs=2) as bpool,
            tc.tile_pool(name="psum", bufs=2, space="PSUM") as ppool,
            tc.tile_pool(name="outp", bufs=2) as opool,
        ):
            iota_i = cpool.tile([P, G * 32], i32)
            nc.gpsimd.iota(
                iota_i[:], pattern=[[0, G], [1, 32]], base=0, channel_multiplier=0
            )
            iota_h = cpool.tile([P, G * 32], f16)
            nc.vector.tensor_copy(iota_h[:], iota_i[:])
            iota3 = iota_h[:].rearrange("p (g j) -> p g j", g=G)

            A = mybir.AluOpType
            for s in range(NSLAB):
                xt = xpool.tile([P, FREE], f32, tag="x")
                nc.sync.dma_start(xt[:], x_d[s])

                hi_h = wpool.tile([P, FREE], f16, tag="hi")
                lo_h = wpool.tile([P, FREE], f16, tag="lo")
                fr_h = wpool.tile([P, FREE], f16, tag="fr")

                for c0 in range(0, FREE, CHUNK):
                    cs = slice(c0, c0 + CHUNK)
                    # idx = round_to_nearest(x - 0.5): int32 write converts
                    # the fp32 ALU result with round-to-nearest.
                    idx_i = wpool.tile([P, CHUNK], i32, tag="idxi")
                    nc.vector.tensor_scalar(
                        out=idx_i[:], in0=xt[:, cs], scalar1=0.5, scalar2=None,
                        op0=A.subtract,
                    )
                    idx_f = wpool.tile([P, CHUNK], f32, tag="idxf")
                    nc.gpsimd.tensor_copy(idx_f[:], idx_i[:])
                    nc.vector.tensor_sub(fr_h[:, cs], xt[:, cs], idx_f[:])
                    hi_i = wpool.tile([P, CHUNK], i32, tag="hii")
                    nc.vector.tensor_scalar(
                        out=hi_i[:], in0=idx_f[:], scalar1=15.5, scalar2=1.0 / 32.0,
                        op0=A.subtract, op1=A.mult,
                    )
                    nc.scalar.copy(hi_h[:, cs], hi_i[:])
                    hi_f = wpool.tile([P, CHUNK], f32, tag="hif")
                    nc.gpsimd.tensor_copy(hi_f[:], hi_i[:])
                    t1 = wpool.tile([P, CHUNK], f32, tag="t1")
                    nc.vector.tensor_scalar(
                        out=t1[:], in0=hi_f[:], scalar1=32.0, scalar2=None,
                        op0=A.mult,
                    )
                    nc.vector.tensor_sub(lo_h[:, cs], idx_f[:], t1[:])

                psum_t = ppool.tile([32 * rot, 64], f32, tag="acc")
                for c in range(FREE // G):
                    gs = slice(c * G, (c + 1) * G)
                    U = bpool.tile([P, G, 32], f16, tag="U")
                    nc.vector.tensor_tensor(
                        out=U[:, :, :],
                        in0=hi_h[:, gs].to_broadcast([P, G, 32]),
                        in1=iota3,
                        op=A.is_equal,
                    )
                    V = bpool.tile([P, G, 64], f16, tag="V")
                    nc.vector.tensor_tensor(
                        out=V[:, :, 0:32],
                        in0=lo_h[:, gs].to_broadcast([P, G, 32]),
                        in1=iota3,
                        op=A.is_equal,
                    )
                    nc.vector.tensor_tensor(
                        out=V[:, :, 32:64],
                        in0=V[:, :, 0:32],
                        in1=fr_h[:, gs].to_broadcast([P, G, 32]),
                        op=A.mult,
                    )
                    for g in range(G):
                        gi = c * G + g
                        q = gi % rot
                        nc.tensor.matmul(
                            out=psum_t[32 * q : 32 * (q + 1), :],
                            lhsT=U[:, g, :],
                            rhs=V[:, g, :],
                            start=(gi < rot),
                            stop=(gi >= FREE - rot),
                        )

                out_s = opool.tile([32 * rot, 64], f32, tag="o")
                nc.scalar.copy(out_s[:], psum_t[:])
                nc.sync.dma_start(out_d[s], out_s[:])

    # Split multi-wait instructions (this walrus accepts at most one
    # sync-wait per instruction; InstEventSemaphore carries two).
    import bass_rust as _bass_rust  # noqa: PLC0415

    _bass_rust.generate_event_semaphores(nc)
    return nc


def _get_program():
    if "nc" not in _program_cache:
        _program_cache["nc"] = _build_program()
    return _program_cache["nc"]


def kernel(bayer_gt: np.ndarray, bayer_out: np.ndarray) -> np.ndarray:
    gt = np.asarray(bayer_gt, dtype=np.float32)
    ot = np.asarray(bayer_out, dtype=np.float32)

    in_maps = []
    for c in range(NCORES):
        slabs = []
        for bl in range(B_PER_CORE):
            b = B_PER_CORE * c + bl
            for arr in (gt, ot):
                for i in (0, 1):
                    for j in (0, 1):
                        slabs.append(arr[b, 0, i::2, j::2])
        x = np.ascontiguousarray(np.stack(slabs)).reshape(NSLAB, P, FREE)
        in_maps.append({"x": x})

    nc = _get_program()
    import os  # noqa: PLC0415

    trace = bool(os.environ.get("KL_TRACE"))
    res = run_bass_kernel_spmd(nc, in_maps, list(range(NCORES)), trace=trace)
    _program_cache["last_results"] = res

    # Host epilogue (float64): reassemble histograms, logs, symmetric KL.
    n = float(NPIX)
    kl_per_phase = np.zeros(4, dtype=np.float64)
    for c in range(NCORES):
        raw = np.asarray(res.results[c]["out"], dtype=np.float64)  # [16, 32*ROT, 64]
        cs = raw.reshape(NSLAB, ROT, 32, 64).sum(axis=1)          # fold quadrants
        C = cs[:, :, :32].reshape(NSLAB, N_BINS)
        S = cs[:, :, 32:].reshape(NSLAB, N_BINS)
        hist = C - S
        hist[:, 1:] += S[:, :-1]
        h = hist / n
        h = np.where(h != 0.0, h, 1.0 / n)
        lh = np.log(h)
        for bl in range(B_PER_CORE):
            for p in range(4):
                sg = bl * 8 + p          # gt slab
                so = bl * 8 + 4 + p      # out slab
                hg, ho = h[sg], h[so]
                lg, lo = lh[sg], lh[so]
                kl = 0.5 * (np.sum(hg * (lg - lo)) + np.sum(ho * (lo - lg)))
                kl_per_phase[p] += kl

    return np.float32(kl_per_phase.mean())



# revision 35
# speedup vs baseline: 3.7915x; 3.7915x over previous
"""Trainium2 kernel for nn_KL_Loss (symmetric KL between triangular histograms
of Bayer phases).

Strategy (relu-ramp scan, multi-engine builds)
-----------------------------
Data-parallel over batch: core c handles batches {2c, 2c+1}; 16 slabs of
512x512 fp32 pixels per core (2 batches x 2 tensors x 4 Bayer phases).

Per slab, bins factor as t = 32*hi + j. For each 128-pixel group the PE
accumulates PSUM[a, col] += onehot32(hi) (x) V where

  V = [1, relu(64 - pos64), relu(pos64 - 64*0), ..., relu(pos64 - 64*31)]

with pos64 = f16(64 * (x - 32*hi)) in [0, 2048]. All relu outputs are exact
in f16 (values share pos64's mantissa bits). The triangular histogram is the
second finite difference of the accumulated ramps (B-spline identity
hat(d) = relu(d+64) - 2 relu(d) + relu(d-64), scaled by 64), reconstructed
host-side in float64:

  64*hist[32a+0]  = sum(relu(64 - pos64))           (anti-ramp column)
  64*hist[32a+j]  = R[a,j-1] - 2 R[a,j] + R[a,j+1]  (j>=1, R[a,32]=0)
  64*hist[32a+32] += R[a,31]                        (carry into next block)

Builds use tensor_scalar scans (one op per output column, 4x DVE mode)
instead of broadcast tensor_tensor one-hots (2x mode): ~2x less DVE time and
a 34-wide moving operand instead of 64.

PSUM rounding noise on the big ramp sums is controlled by accumulating each
slab in 8 window tiles of 256 groups each (separate PSUM banks), summed in
f64 on the host along with the rest of the KL math.
"""

import sys

sys.path.insert(0, "/opt/trn_rl_repo")

import numpy as np

import concourse.mybir as mybir
import concourse.tile as tile_mod
from concourse import bass
from concourse.bass_utils import run_bass_kernel_spmd
from concourse.vector_clock import ScopedClock

# ---------------------------------------------------------------------------
# Workaround for this walrus build: instructions carry at most one sync-wait
# command, but TileContext's exit path piles every outstanding proc-clock wait
# onto the single final Drain -> "Too many sync wait commands". Redistribute
# them one-per-nop before the drain.
# ---------------------------------------------------------------------------


def _split_drain_and_barrier(self, tick_clock, wait_clock):
    nc = self.nc
    collector = nc.sync.nop(nofuse=True, hint="drain_wait_split")
    wait_clock.add_sem_waits(
        collector.ins, ScopedClock({None: tick_clock.global_clock})
    )
    si = collector.ins.sync_info
    waits = list(si.on_wait) if si is not None else []
    if len(waits) > 1:
        collector.ins.sync_info = mybir.SyncInfo(
            on_wait=[waits[0]], on_update=list(si.on_update)
        )
        for w in waits[1:]:
            n = nc.sync.nop(nofuse=True, hint="drain_wait_split")
            n.ins.sync_info = mybir.SyncInfo(on_wait=[w], on_update=[])

    nc.sync.drain()
    nc.all_engine_barrier()
    assert self.sems is not None
    popped = nc._tile_sem_poison_stack.pop()
    assert popped is self._sem_poison
    nc.clear_and_free_semaphores(list(self.sems.allocated().values()))
    nc.all_engine_barrier()


tile_mod.TileContext._drain_and_barrier = _split_drain_and_barrier

# ---------------------------------------------------------------------------
# Problem constants (hardcoded per spec: inputs (16,1,1024,1024) fp32).
# ---------------------------------------------------------------------------
B = 16
H = W = 1024
NCORES = 8
B_PER_CORE = B // NCORES
NSLAB = B_PER_CORE * 2 * 4          # 16 slabs per core
PH, PW = H // 2, W // 2             # 512 x 512 per phase
NPIX = PH * PW                      # 262144 pixels per slab
P = 128
FREE = NPIX // P                    # 2048 pixel-groups (columns) per slab
CHUNK = 512                         # build chunk along the group axis
NBLK = 32                           # hi alphabet (PSUM rows)
NJ = 32                             # ramp columns r_0..r_31
VC = 2 + NJ                         # V cols: [ones, min(pos,64), r_j...]
SCALE = 64.0                        # pos scaling (frac quantum 1/64 bin)
NWIN = 8                            # PSUM window tiles per slab
WGRP = FREE // NWIN                 # 256 groups per window
N_BINS = 1024

f32 = mybir.dt.float32
f16 = mybir.dt.float16
i32 = mybir.dt.int32

_program_cache = {}


ACT_COLS = tuple(range(6, 24))       # relu cols built on ScalarE (18)
GPS_COLS = ()                        # GPSIMD: ~8us/instruction, never use
# remaining relu cols + U scans + ones + prep stay on VectorE


def _build_program(nslab=NSLAB, chunk=None, uv_bufs=2, nonce=0, do_builds=True, do_mm=True, act_cols=ACT_COLS, gps_cols=GPS_COLS, u_tt=False, nrep=1):
    nc = bass.Bass()
    # Distinct nonce shapes force distinct XLA module hashes: the NEFF cache
    # key does not cover the embedded BIR, so shape-identical program
    # variants would otherwise silently reuse each other's binaries.
    nonce_d = nc.declare_dram_parameter("nonce", [1, 8 + nonce], f32, isOutput=False)
    x_d = nc.declare_dram_parameter("x", [NSLAB, P, FREE], f32, isOutput=False)
    out_d = nc.declare_dram_parameter(
        "out", [NSLAB, NBLK, NWIN, VC], f32, isOutput=True
    )

    A = mybir.AluOpType
    CH = CHUNK if chunk is None else chunk

    with tile_mod.TileContext(nc) as tc:
        with (
            tc.tile_pool(name="nn", bufs=1) as npool,
            tc.tile_pool(name="xin", bufs=2) as xpool,
            tc.tile_pool(name="work", bufs=uv_bufs) as wpool,
            tc.tile_pool(name="uv", bufs=uv_bufs) as uvpool,
            tc.tile_pool(name="psum", bufs=2, space="PSUM") as ppool,
            tc.tile_pool(name="outp", bufs=2) as opool,
        ):
            nt = npool.tile([1, 8 + nonce], f32)
            nc.sync.dma_start(nt[:], nonce_d[:])

            CH_ = CHUNK if chunk is None else chunk
            iota_a = npool.tile([P, NBLK, CH_], f16)
            bias_t = npool.tile([P, NJ + 1], f32)
            with tc.tile_pool(name="tmp", bufs=1) as tmpool:
                iota_s = tmpool.tile([P, NBLK], i32)
                nc.gpsimd.iota(
                    iota_s[:], pattern=[[1, NBLK]], base=0, channel_multiplier=0
                )
                iota_sf = tmpool.tile([P, NBLK], f16)
                nc.vector.tensor_copy(iota_sf[:], iota_s[:])
                nc.vector.tensor_copy(
                    iota_a[:],
                    iota_sf[:].rearrange("p a -> p a ()").to_broadcast(
                        [P, NBLK, CH_]
                    ),
                )
                bias_i = tmpool.tile([P, NJ + 1], i32)
                nc.gpsimd.iota(
                    bias_i[:], pattern=[[-int(SCALE), NJ + 1]], base=int(SCALE),
                    channel_multiplier=0,
                )
                nc.vector.tensor_copy(bias_t[:], bias_i[:])

            for s_ in range(nslab * nrep):
                s = s_ % nslab
                xt = xpool.tile([P, FREE], f32, tag="x")
                nc.sync.dma_start(xt[:], x_d[s])

                out_s = opool.tile([NBLK, NWIN, VC], f32, tag="o")

                psum_t = ppool.tile([NBLK, NWIN, VC], f32, tag="acc")

                for ci, c0 in enumerate(range(0, FREE, CH)):
                    cs = slice(c0, c0 + CH)
                    # ---- prep: hi = round((x-16)/32); pos64 = 64*(x-32*hi)
                    hi_i = wpool.tile([P, CH], i32, tag="hii")
                    nc.vector.tensor_scalar(
                        out=hi_i[:], in0=xt[:, cs], scalar1=16.0,
                        scalar2=1.0 / 32.0, op0=A.subtract, op1=A.mult,
                    )
                    hi16 = wpool.tile([P, CH], f16, tag="hi16")
                    nc.scalar.copy(hi16[:], hi_i[:])
                    hi32s = wpool.tile([P, CH], f32, tag="hi32s")
                    nc.scalar.mul(hi32s[:], hi_i[:], 32.0)
                    pos_f = wpool.tile([P, CH], f32, tag="posf")
                    nc.vector.tensor_sub(pos_f[:], xt[:, cs], hi32s[:])
                    pos64 = wpool.tile([P, CH], f16, tag="pos64")
                    nc.vector.tensor_scalar(
                        out=pos64[:], in0=pos_f[:], scalar1=SCALE, scalar2=None,
                        op0=A.mult,
                    )

                    # ---- U tile: onehot32(hi) via one broadcast is_equal
                    U = uvpool.tile([P, NBLK, CH], f16, tag="U")
                    if do_builds and u_tt:
                        nc.vector.tensor_tensor(
                            out=U[:, :, :],
                            in0=hi16[:].rearrange("p c -> p () c").to_broadcast(
                                [P, NBLK, CH]
                            ),
                            in1=iota_a[:, :, :CH],
                            op=A.is_equal,
                        )
                    elif do_builds:
                        for a in range(NBLK):
                            nc.vector.tensor_scalar(
                                out=U[:, a, :], in0=hi16[:], scalar1=float(a),
                                scalar2=None, op0=A.is_equal,
                            )

                    # ---- V tile: [ones, relu(64-pos), relu(pos-64j)...]
                    # (col 1 = anti-ramp: its sum IS 64*hist[j=0] directly)
                    V = uvpool.tile([P, VC, CH], f16, tag="V")
                    nc.vector.memset(V[:, 0, :], 1.0)
                    nc.scalar.activation(
                        V[:, 1, :], pos64[:],
                        mybir.ActivationFunctionType.Relu,
                        bias=bias_t[:, 0:1], scale=-1.0,
                    )
                    for j in range(NJ if do_builds else 0):
                        if j in act_cols:
                            nc.scalar.activation(
                                V[:, 2 + j, :], pos64[:],
                                mybir.ActivationFunctionType.Relu,
                                bias=bias_t[:, j + 1 : j + 2], scale=1.0,
                            )
                        elif j in gps_cols:
                            nc.gpsimd.tensor_scalar(
                                out=V[:, 2 + j, :], in0=pos64[:],
                                scalar1=SCALE * j, scalar2=0.0,
                                op0=A.subtract, op1=A.max,
                            )
                        else:
                            nc.vector.tensor_scalar(
                                out=V[:, 2 + j, :], in0=pos64[:],
                                scalar1=SCALE * j, scalar2=0.0,
                                op0=A.subtract, op1=A.max,
                            )

                    # ---- matmuls: one LDW+MM per 128-pixel group
                    for g in range(CH if do_mm else 0):
                        gg = c0 + g
                        w = gg // WGRP
                        nc.tensor.matmul(
                            out=psum_t[:, w, :],
                            lhsT=U[:, :, g],
                            rhs=V[:, :, g],
                            start=(gg % WGRP == 0),
                            stop=(gg % WGRP == WGRP - 1),
                        )

                if do_mm:
                    nc.scalar.copy(out_s[:], psum_t[:])
                else:
                    nc.vector.memset(out_s[:], 0.0)
                nc.sync.dma_start(out_d[s], out_s[:])

    import bass_rust  # noqa: PLC0415

    bass_rust.generate_event_semaphores(nc)
    return nc


def _get_program():
    if "nc" not in _program_cache:
        _program_cache["nc"] = _build_program()
    return _program_cache["nc"]


def _hists_from_raw(raw):
    """raw: [NSLAB, 32, NWIN, 34] f32 -> [NSLAB, 1024] f64 unnormalized."""
    R = raw.astype(np.float64).sum(axis=2)          # [NSLAB, 32, 34]
    N = R[:, :, 0]                                  # [NSLAB, 32]
    C0 = R[:, :, 1]
    r = R[:, :, 2:]                                 # [NSLAB, 32, 32]
    ns = R.shape[0]
    hist = np.zeros((ns, N_BINS + 1), np.float64)
    # j = 0 bins: col 1 is the anti-ramp sum(relu(64 - pos64)) = 64*hat0
    hist[:, 0::32][:, :32] = C0
    # j >= 1: second difference of ramp sums
    rpad = np.concatenate([r, np.zeros((ns, 32, 1))], axis=2)  # r[a, 32] = 0
    for j in range(1, 32):
        hist[:, j::32][:, :32] = (
            rpad[:, :, j - 1] - 2.0 * rpad[:, :, j] + rpad[:, :, j + 1]
        )
    # carry: hat at j = 32 of block a -> bin 32(a+1)
    hist[:, 32::32] += r[:, :, 31]
    # bin 1024 (only f16-edge dust) -> fold into 1023
    hist[:, 1023] += hist[:, 1024]
    return hist[:, :N_BINS] / SCALE


def kernel(bayer_gt: np.ndarray, bayer_out: np.ndarray) -> np.ndarray:
    gt = np.asarray(bayer_gt, dtype=np.float32)
    ot = np.asarray(bayer_out, dtype=np.float32)

    in_maps = []
    for c in range(NCORES):
        slabs = []
        for bl in range(B_PER_CORE):
            b = B_PER_CORE * c + bl
            for arr in (gt, ot):
                for i in (0, 1):
                    for j in (0, 1):
                        slabs.append(arr[b, 0, i::2, j::2])
        x = np.ascontiguousarray(np.stack(slabs)).reshape(NSLAB, P, FREE)
        in_maps.append({"x": x, "nonce": np.zeros((1, 8), np.float32)})

    nc = _get_program()
    import os  # noqa: PLC0415

    trace = bool(os.environ.get("KL_TRACE"))
    res = run_bass_kernel_spmd(nc, in_maps, list(range(NCORES)), trace=trace)
    _program_cache["last_results"] = res

    # Host epilogue (float64): histograms, logs, symmetric KL.
    n = float(NPIX)
    kl_per_phase = np.zeros(4, dtype=np.float64)
    for c in range(NCORES):
        raw = np.asarray(res.results[c]["out"], dtype=np.float64)
        hist = _hists_from_raw(raw)
        h = hist / n
        h = np.where(h != 0.0, h, 1.0 / n)
        lh = np.log(h)
        for bl in range(B_PER_CORE):
            for p in range(4):
                sg = bl * 8 + p          # gt slab
                so = bl * 8 + 4 + p      # out slab
                hg, ho = h[sg], h[so]
                lg, lo = lh[sg], lh[so]
                kl = 0.5 * (np.sum(hg * (lg - lo)) + np.sum(ho * (lo - lg)))
                kl_per_phase[p] += kl

    return np.float32(kl_per_phase.mean())
